# revision 17
# baseline (speedup 1.0000x reference)
"""Trainium2 Bass kernel for nn_ActorTorsionNet (GNN message passing).

Sharding: dst-sorted edges packed into 128-edge/48-node-slot blocks; nodes in
8 contiguous shards balanced by edge count. Per-edge [64,64] weights (We)
materialized once to HBM in bf16 with (f,d) column order, streamed back each
of the 6 GRU steps; DVE multiply + add-tree reduce produce messages, and the
scatter-mean is TensorE matmuls against host-built inv_deg-weighted selection
matrices (root/bias terms accumulated in the same PSUM tile). Node features
are d-major [64, nloc]; an AllGather replicates them each step for the edge
gather. Set2Set pooling is replicated; the LSTM head is sharded by torsion.
All per-core differences flow through input tensors (single SPMD graph).
"""
import numpy as np

DIM = 64
N_CORES = 8
EBLK = 128      # edges per block
VBLK = 56       # node slots per block
NCHUNK = 512    # free-dim chunk for node-wise matmuls
ITERS = 6       # GRU iterations (debug knob)


# --------------------------------------------------------------------------
# host-side index prep
# --------------------------------------------------------------------------

def _host_prep(edge_index, n_nodes):
    src = np.asarray(edge_index[0]).astype(np.int64)
    dst = np.asarray(edge_index[1]).astype(np.int64)
    E = src.shape[0]

    deg = np.bincount(dst, minlength=n_nodes)
    inv_deg = (1.0 / np.maximum(deg, 1.0)).astype(np.float32)

    order = np.argsort(dst, kind="stable")
    src_s = src[order]

    ptr = np.zeros(n_nodes + 1, np.int64)
    np.cumsum(deg, out=ptr[1:])

    bounds = [0]
    for c in range(1, N_CORES):
        target = E * c // N_CORES
        bounds.append(int(np.searchsorted(ptr, target, side="left")))
    bounds.append(n_nodes)

    cores = []
    for c in range(N_CORES):
        segs = [(int(v), int(ptr[v]), int(ptr[v + 1]))
                for v in range(bounds[c], bounds[c + 1])]
        segs.sort(key=lambda s: -(s[2] - s[1]))
        blocks = []
        for v, a, b in segs:
            ne = b - a
            for blk in blocks:
                if blk["ne"] + ne <= EBLK and len(blk["segs"]) < VBLK:
                    blk["segs"].append((v, a, b))
                    blk["ne"] += ne
                    break
            else:
                blocks.append({"segs": [(v, a, b)], "ne": ne})
        cores.append(blocks)

    B = max(len(bl) for bl in cores)
    nloc = ((B * VBLK + NCHUNK - 1) // NCHUNK) * NCHUNK
    npad = N_CORES * nloc
    E_shard = B * EBLK

    node_slot = np.full(n_nodes, -1, np.int64)
    edge_id = np.full((N_CORES, E_shard), -1, np.int64)
    gather_src = np.zeros((N_CORES, E_shard), np.int64)
    sel_w = np.zeros((N_CORES, EBLK, B, VBLK), np.float32)
    for c in range(N_CORES):
        for b, blk in enumerate(cores[c]):
            ei = 0
            for nv, (v, a, bb) in enumerate(blk["segs"]):
                node_slot[v] = c * nloc + b * VBLK + nv
                w = inv_deg[v]
                for k in range(a, bb):
                    j = b * EBLK + ei
                    edge_id[c, j] = order[k]
                    gather_src[c, j] = src_s[k]
                    sel_w[c, ei, b, nv] = w
                    ei += 1
    assert (node_slot >= 0).all()
    gather_idx = node_slot[gather_src]

    return dict(B=B, nloc=nloc, npad=npad, E_shard=E_shard, bounds=bounds,
                node_slot=node_slot, edge_id=edge_id, gather_idx=gather_idx,
                sel_w=sel_w)


def _wrap16(idx):
    """dma_gather index layout: value i at [i % 16, i // 16]."""
    idx = np.asarray(idx, np.int64)
    n = idx.shape[0]
    assert n % 16 == 0
    out = np.zeros((16, n // 16), np.int16)
    out[np.arange(n) % 16, np.arange(n) // 16] = idx.astype(np.int16)
    return out


# --------------------------------------------------------------------------
# device kernel builder
# --------------------------------------------------------------------------

def _build(B, nloc, npad):
    import concourse.bacc as bacc
    import concourse.mybir as mybir
    from concourse import bass_isa, tile

    f32 = mybir.dt.float32
    bf16 = mybir.dt.bfloat16
    i16 = mybir.dt.int16
    AF = mybir.ActivationFunctionType
    ALU = mybir.AluOpType
    AX = mybir.AxisListType
    E = B * EBLK
    NLT = nloc // 128
    NT = npad // 128
    NCH = nloc // NCHUNK
    RG = [list(range(N_CORES))]

    nc = bacc.Bacc(debug=False)

    def par(name, shape, dtype=f32):
        return nc.declare_dram_parameter(name, list(shape), dtype, isOutput=False)

    ea_t = par("ea_t", [7, E])
    we1_t = par("we1_t", [7, 128])
    be1 = par("be1", [128, 1])
    we2p_t = par("we2p_t", [128, 4096])
    be2r = par("be2r", [64, 64])
    x_t = par("x_t", [3, nloc])
    w0m = par("w0m", [3, 64])
    b0 = par("b0", [64, 1])
    root_p = par("root", [64, 64])
    conv_b = par("conv_b", [64, 1])
    wihT = par("wihT", [64, 192])
    whhT = par("whhT", [64, 192])
    grb = par("grb", [64, 1])
    gzb = par("gzb", [64, 1])
    bihn = par("bihn", [64, 1])
    bhhn = par("bhhn", [64, 1])
    sel = par("sel", [128, B * VBLK])
    gidx = par("gidx", [16, E // 16], i16)
    hidx = par("hidx", [16, 32], i16)
    s2s_wiT = par("s2s_wiT", [64, 512])
    s2s_whT = par("s2s_whT", [64, 256])
    s2s_b = par("s2s_b", [64, 4])
    smask = par("smask", [128, NT])
    wmiT = par("wmiT", [128, 3 * 1536])
    mbias = par("mbias", [128, 12])
    w1T = par("w1T", [128, 3 * 128])
    b1 = par("b1", [128, 1])
    w2T = par("w2T", [128, 6])
    b2 = par("b2", [6, 1])
    idn = par("idn", [128, 128])
    psel = par("psel", [64, 32])
    out_p = nc.declare_dram_parameter("out", [128, 6], f32, isOutput=True)

    with tile.TileContext(nc) as tc:
        with (
            tc.tile_pool(name="cst", bufs=1) as cst,
            tc.tile_pool(name="st", bufs=1) as st,
            tc.tile_pool(name="psA", bufs=2, space="PSUM") as psA,
            tc.tile_pool(name="psB", bufs=2, space="PSUM") as psB,
            tc.tile_pool(name="psC", bufs=4, space="PSUM") as psC,
            tc.tile_pool(name="dram", bufs=1, space="DRAM") as dram,
        ):
            def pa_t(shape, dtype=f32):
                return psA.tile(list(shape), dtype, tag="psa", name="psa")

            def pb_t(shape, dtype=f32):
                return psB.tile(list(shape), dtype, tag="psb", name="psb")

            def pc_t(shape, dtype=f32):
                return psC.tile(list(shape), dtype, tag="psc", name="psc")

            # ----- persistent constants (small, bf16 where possible) -----
            def loadc(pool, p, shape, dtype=f32, tag=None):
                t = pool.tile(list(shape), dtype, tag=tag or ("ld_" + p.name), name="ld_" + p.name)
                nc.sync.dma_start(t[:], p[:])
                return t

            be1_sb = loadc(cst, be1, [128, 1])
            b0_sb = loadc(cst, b0, [64, 1])
            conv_sb = loadc(cst, conv_b, [64, 1])
            grb_sb = loadc(cst, grb, [64, 1])
            gzb_sb = loadc(cst, gzb, [64, 1])
            bihn_sb = loadc(cst, bihn, [64, 1])
            bhhn_sb = loadc(cst, bhhn, [64, 1])
            gidx_sb = cst.tile([128, E // 16], i16, tag="gidx_sb")
            nc.vector.memset(gidx_sb[:], 0)
            nc.sync.dma_start(gidx_sb[0:16, :], gidx[:])
            idn_sb = loadc(cst, idn, [128, 128])
            idn_bf = cst.tile([128, 128], bf16, tag="idn_bf")
            nc.vector.tensor_copy(idn_bf[:], idn_sb[:])
            ones_bf = cst.tile([1, 128], bf16, tag="ones_bf")
            nc.vector.memset(ones_bf[:], 1.0)
            ones_f = cst.tile([1, 128], f32, tag="ones_f")
            nc.vector.memset(ones_f[:], 1.0)

            sel_bf = cst.tile([128, B * VBLK], bf16, tag="sel_bf")
            root_bf = cst.tile([64, 64], bf16, tag="root_bf")
            be2r_bf = cst.tile([64, 64], bf16, tag="be2r_bf")
            wihT_bf = cst.tile([64, 192], bf16, tag="wihT_bf")
            whhT_bf = cst.tile([64, 192], bf16, tag="whhT_bf")

            # ----- persistent state -----
            h_t = st.tile([64, nloc], f32, tag="h_t")
            nc.vector.memset(h_t[:], 0.0)
            out_bf = st.tile([64, nloc], bf16, tag="out_bf")
            m_bf = st.tile([64, nloc], bf16, tag="m_bf")
            nc.vector.memset(m_bf[:], 0.0)
            s_gat = st.tile([128, B, 64], f32, tag="s_gat")
            s_bf = st.tile([128, B, 64], bf16, tag="s_bf")

            # ----- internal DRAM -----
            we_dram = dram.tile([E, 4096], bf16)
            ag_ins = []
            ag_outs = []
            for i in range(ITERS + 1):
                agi = dram.tile([nloc, 64], f32, name=f"ag_in{i}", tag=f"agi{i}")
                ago = dram.tile([npad, 64], f32, addr_space="Shared",
                                name=f"ag_out{i}", tag=f"ago{i}")
                ag_ins.append(agi)
                ag_outs.append(ago)

            # =============== phase 0: staging + edge MLP -> We ===============
            with tc.tile_pool(name="ph0", bufs=1) as ph0:
                def stage_cast(p, shape, dst):
                    t = ph0.tile(list(shape), f32, tag="stage", name="stage")
                    nc.sync.dma_start(t[:], p[:])
                    nc.vector.tensor_copy(dst[:], t[:])

                stage_cast(sel, [128, B * VBLK], sel_bf)
                stage_cast(root_p, [64, 64], root_bf)
                stage_cast(be2r, [64, 64], be2r_bf)
                stage_cast(wihT, [64, 192], wihT_bf)
                stage_cast(whhT, [64, 192], whhT_bf)

                ea_sb = loadc(ph0, ea_t, [7, E])
                we1_sb = loadc(ph0, we1_t, [7, 128])
                x_sb = loadc(ph0, x_t, [3, nloc])
                w0m_sb = loadc(ph0, w0m, [3, 64])
                we2p_sb = loadc(ph0, we2p_t, [128, 4096])
                we2p_bf = ph0.tile([128, 4096], bf16, tag="we2p_bf")
                nc.vector.tensor_copy(we2p_bf[:], we2p_sb[:])

                h_edge = ph0.tile([128, E], bf16, tag="h_edge")
                for ch in range(E // NCHUNK):
                    ps = pc_t([128, NCHUNK])
                    nc.tensor.matmul(ps[:], we1_sb[:],
                                     ea_sb[:, ch * NCHUNK:(ch + 1) * NCHUNK],
                                     start=True, stop=True)
                    nc.scalar.activation(h_edge[:, ch * NCHUNK:(ch + 1) * NCHUNK],
                                         ps[:], AF.Relu, bias=be1_sb[:])

                with tc.tile_pool(name="wes", bufs=2) as wes:
                    for b in range(B):
                        we_sb = wes.tile([128, 4096], bf16, tag="wesb")
                        for j in range(8):
                            ps = pc_t([128, 512])
                            nc.tensor.matmul(
                                ps[:], h_edge[:, b * 128:(b + 1) * 128],
                                we2p_bf[:, j * 512:(j + 1) * 512],
                                start=True, stop=True)
                            if j % 2 == 0:
                                nc.vector.tensor_copy(
                                    we_sb[:, j * 512:(j + 1) * 512], ps[:])
                            else:
                                nc.scalar.copy(
                                    we_sb[:, j * 512:(j + 1) * 512], ps[:])
                        nc.sync.dma_start(we_dram[b * 128:(b + 1) * 128, :],
                                          we_sb[:])

                # lin0 (inside ph0 scope: uses x_sb)
                for ch in range(NCH):
                    sl = slice(ch * NCHUNK, (ch + 1) * NCHUNK)
                    ps = pc_t([64, NCHUNK])
                    nc.tensor.matmul(ps[:], w0m_sb[:], x_sb[:, sl],
                                     start=True, stop=True)
                    nc.scalar.activation(h_t[:, sl], ps[:], AF.Relu, bias=b0_sb[:])

            # =============== 6 GRU iterations ===============
            with (
                tc.tile_pool(name="wk", bufs=3) as wk,
                tc.tile_pool(name="tr", bufs=2) as tr,
            ):
                def emit_allgather(ag_in, ag_out):
                    onm = wk.tile([128, NLT * 64], f32, tag="onm")
                    for t0 in range(0, NLT, 8):
                        grp = min(8, NLT - t0)
                        ps = pc_t([128, 512])
                        for k in range(grp):
                            t = t0 + k
                            nc.tensor.transpose(ps[:, k * 64:(k + 1) * 64],
                                                h_t[:, t * 128:(t + 1) * 128],
                                                idn_sb[0:64, 0:64])
                        nc.scalar.copy(onm[:, t0 * 64:(t0 + grp) * 64],
                                       ps[:, 0:grp * 64])
                    nc.sync.dma_start(
                        ag_in[:].rearrange("(p t) d -> p t d", p=128),
                        onm[:].rearrange("p (t d) -> p t d", d=64))
                    nc.gpsimd.collective_compute(
                        "AllGather", ALU.bypass, replica_groups=RG,
                        ins=[ag_in[:].opt()], outs=[ag_out[:].opt()])

                def tree_reduce(eng, tmp, sc1, F, msg_out):
                    """Ping-pong pairwise adds over trailing 64 -> msg_out."""
                    nc_e = eng
                    nc_e.tensor_add(sc1[:, :, 0:32], tmp[:, :, 0:32],
                                    tmp[:, :, 32:64])
                    nc_e.tensor_add(tmp[:, :, 0:16], sc1[:, :, 0:16],
                                    sc1[:, :, 16:32])
                    nc_e.tensor_add(sc1[:, :, 0:8], tmp[:, :, 0:8],
                                    tmp[:, :, 8:16])
                    nc_e.tensor_add(tmp[:, :, 0:4], sc1[:, :, 0:4],
                                    sc1[:, :, 4:8])
                    nc_e.tensor_add(sc1[:, :, 0:2], tmp[:, :, 0:2],
                                    tmp[:, :, 2:4])
                    nc_e.tensor_add(msg_out[:], sc1[:, :, 0:1].squeeze(2),
                                    sc1[:, :, 1:2].squeeze(2))

                for it in range(ITERS):
                    ag_out = ag_outs[it]
                    nc.vector.tensor_copy(out_bf[:], h_t[:])
                    emit_allgather(ag_ins[it], ag_out)
                    nc.gpsimd.dma_gather(s_gat[:], ag_out[:], gidx_sb[:],
                                         num_idxs=E, num_idxs_reg=E,
                                         elem_size=64, single_packet=False)
                    nc.vector.tensor_copy(s_bf[:], s_gat[:])

                    for b in range(B):
                        we_sb = wk.tile([128, 64, 64], bf16, tag="westream")
                        nc.sync.dma_start(
                            we_sb[:].rearrange("p a b -> p (a b)"),
                            we_dram[b * 128:(b + 1) * 128, :])
                        tmp = tr.tile([128, 64, 64], bf16, tag="tmp")
                        sc1 = tr.tile([128, 64, 32], bf16, tag="sc1")
                        sb_b = s_bf[:, b, :].unsqueeze(1).broadcast_to(
                            [128, 64, 64])
                        nc.vector.tensor_mul(tmp[:], we_sb[:], sb_b)
                        msg = tr.tile([128, 64], bf16, tag="msg")
                        eng = nc.gpsimd if (b % 5) < 2 else nc.vector
                        tree_reduce(eng, tmp, sc1, 64, msg)

                        vsl = slice(b * VBLK, (b + 1) * VBLK)
                        pa = pa_t([64, VBLK])
                        nc.tensor.matmul(pa[:], s_bf[:, b, :], sel_bf[:, vsl],
                                         start=True, stop=True)
                        ssum_bf = wk.tile([64, VBLK], bf16, tag="ssum")
                        nc.scalar.copy(ssum_bf[:], pa[:])
                        pb = pb_t([64, VBLK])
                        nc.tensor.matmul(pb[:], be2r_bf[:], ssum_bf[:],
                                         start=True, stop=False)
                        nc.tensor.matmul(pb[:], msg[:], sel_bf[:, vsl],
                                         start=False, stop=False)
                        nc.tensor.matmul(pb[:], root_bf[:], out_bf[:, vsl],
                                         start=False, stop=True)
                        nc.scalar.activation(m_bf[:, vsl], pb[:], AF.Relu,
                                             bias=conv_sb[:])

                    for ch in range(NCH):
                        sl = slice(ch * NCHUNK, (ch + 1) * NCHUNK)
                        pr_ = pc_t([64, NCHUNK])
                        nc.tensor.matmul(pr_[:], wihT_bf[:, 0:64], m_bf[:, sl],
                                         start=True, stop=False)
                        nc.tensor.matmul(pr_[:], whhT_bf[:, 0:64],
                                         out_bf[:, sl], start=False, stop=True)
                        pz_ = pc_t([64, NCHUNK])
                        nc.tensor.matmul(pz_[:], wihT_bf[:, 64:128],
                                         m_bf[:, sl], start=True, stop=False)
                        nc.tensor.matmul(pz_[:], whhT_bf[:, 64:128],
                                         out_bf[:, sl], start=False, stop=True)
                        pxn = pc_t([64, NCHUNK])
                        nc.tensor.matmul(pxn[:], wihT_bf[:, 128:192],
                                         m_bf[:, sl], start=True, stop=True)
                        phn = pc_t([64, NCHUNK])
                        nc.tensor.matmul(phn[:], whhT_bf[:, 128:192],
                                         out_bf[:, sl], start=True, stop=True)
                        r_sb = wk.tile([64, NCHUNK], f32, tag="r_sb")
                        nc.scalar.activation(r_sb[:], pr_[:], AF.Sigmoid,
                                             bias=grb_sb[:])
                        z_sb = wk.tile([64, NCHUNK], f32, tag="z_sb")
                        nc.scalar.activation(z_sb[:], pz_[:], AF.Sigmoid,
                                             bias=gzb_sb[:])
                        ghn = wk.tile([64, NCHUNK], f32, tag="ghn")
                        nc.scalar.activation(ghn[:], phn[:], AF.Identity,
                                             bias=bhhn_sb[:])
                        t1 = wk.tile([64, NCHUNK], f32, tag="t1")
                        nc.vector.tensor_mul(t1[:], r_sb[:], ghn[:])
                        t2 = wk.tile([64, NCHUNK], f32, tag="t2")
                        nc.vector.tensor_add(t2[:], t1[:], pxn[:])
                        nn = wk.tile([64, NCHUNK], f32, tag="nn")
                        nc.scalar.activation(nn[:], t2[:], AF.Tanh,
                                             bias=bihn_sb[:])
                        t3 = wk.tile([64, NCHUNK], f32, tag="t3")
                        nc.vector.tensor_sub(t3[:], h_t[:, sl], nn[:])
                        t4 = wk.tile([64, NCHUNK], f32, tag="t4")
                        nc.vector.tensor_mul(t4[:], z_sb[:], t3[:])
                        nc.vector.tensor_add(h_t[:, sl], nn[:], t4[:])

                emit_allgather(ag_ins[ITERS], ag_outs[ITERS])

            # =============== Set2Set + head ===============
            with tc.tile_pool(name="s2s", bufs=1) as sp:
                def stage_cast2(p, shape, dtype, tag):
                    t = sp.tile(list(shape), f32, tag="stage2", name="stage2")
                    nc.sync.dma_start(t[:], p[:])
                    o = sp.tile(list(shape), dtype, tag=tag, name=tag)
                    nc.vector.tensor_copy(o[:], t[:])
                    return o

                s2s_wiT_bf = stage_cast2(s2s_wiT, [64, 512], bf16, "wi_bf")
                s2s_whT_bf = stage_cast2(s2s_whT, [64, 256], bf16, "wh_bf")
                smask_bf = stage_cast2(smask, [128, NT], bf16, "smask_bf")
                wmiT_bf = stage_cast2(wmiT, [128, 3 * 1536], bf16, "wmiT_bf")
                w1T_bf = stage_cast2(w1T, [128, 3 * 128], bf16, "w1T_bf")
                w2T_bf = stage_cast2(w2T, [128, 6], bf16, "w2T_bf")
                s2s_b_sb = loadc(sp, s2s_b, [64, 4])
                mbias_sb = loadc(sp, mbias, [128, 12])
                b1_sb = loadc(sp, b1, [128, 1])
                b2_sb = loadc(sp, b2, [6, 1])
                hidx_sb = sp.tile([128, 32], i16, tag="hidx_sb")
                nc.vector.memset(hidx_sb[:], 0)
                nc.sync.dma_start(hidx_sb[0:16, :], hidx[:])

                ag_out = ag_outs[ITERS]
                onm_bf = sp.tile([128, NT, 64], bf16, tag="onm_bf")
                nc.gpsimd.dma_start(
                    onm_bf[:].rearrange("p (c t) d -> p c t d", c=N_CORES),
                    ag_out[:].rearrange("(c p t) d -> p c t d",
                                        c=N_CORES, p=128))
                mb = smask_bf[:].unsqueeze(2).broadcast_to([128, NT, 64])
                nc.vector.tensor_mul(onm_bf[:], onm_bf[:], mb)

                q_lo = sp.tile([64, 1], f32, tag="q_lo")
                nc.vector.memset(q_lo[:], 0.0)
                q_hi = sp.tile([64, 1], f32, tag="q_hi")
                nc.vector.memset(q_hi[:], 0.0)
                hs = sp.tile([64, 1], f32, tag="hs")
                nc.vector.memset(hs[:], 0.0)
                cs = sp.tile([64, 1], f32, tag="cs")
                nc.vector.memset(cs[:], 0.0)

                eprod = sp.tile([128, NT, 64], bf16, tag="eprod")
                esc = sp.tile([128, NT, 32], bf16, tag="esc")

                for step in range(6):
                    ql_bf = sp.tile([64, 1], bf16, tag="ql_bf")
                    nc.vector.tensor_copy(ql_bf[:], q_lo[:])
                    qh_bf = sp.tile([64, 1], bf16, tag="qh_bf")
                    nc.vector.tensor_copy(qh_bf[:], q_hi[:])
                    hs_bf = sp.tile([64, 1], bf16, tag="hs_bf")
                    nc.vector.tensor_copy(hs_bf[:], hs[:])
                    gt = []
                    for g, fn in enumerate([AF.Sigmoid, AF.Sigmoid,
                                            AF.Tanh, AF.Sigmoid]):
                        pg = pa_t([64, 1])
                        gsl = slice(g * 64, (g + 1) * 64)
                        nc.tensor.matmul(pg[:], s2s_wiT_bf[:, gsl], ql_bf[:],
                                         start=True, stop=False)
                        nc.tensor.matmul(pg[:],
                                         s2s_wiT_bf[:, 256 + g * 64:
                                                    256 + (g + 1) * 64],
                                         qh_bf[:], start=False, stop=False)
                        nc.tensor.matmul(pg[:], s2s_whT_bf[:, gsl], hs_bf[:],
                                         start=False, stop=True)
                        gv = sp.tile([64, 1], f32, tag=f"gate{g}", name=f"gate{g}")
                        nc.scalar.activation(gv[:], pg[:], fn,
                                             bias=s2s_b_sb[:, g:g + 1])
                        gt.append(gv)
                    t5 = sp.tile([64, 1], f32, tag="t5")
                    nc.vector.tensor_mul(t5[:], gt[1][:], cs[:])
                    t6 = sp.tile([64, 1], f32, tag="t6")
                    nc.vector.tensor_mul(t6[:], gt[0][:], gt[2][:])
                    nc.vector.tensor_add(cs[:], t5[:], t6[:])
                    tch = sp.tile([64, 1], f32, tag="tch")
                    nc.scalar.activation(tch[:], cs[:], AF.Tanh)
                    nc.vector.tensor_mul(hs[:], gt[3][:], tch[:])

                    hsb2 = sp.tile([64, 1], bf16, tag="hsb2")
                    nc.vector.tensor_copy(hsb2[:], hs[:])
                    pq = pa_t([1, 64], bf16)
                    nc.tensor.transpose(pq[:], hsb2[:], idn_bf[0:64, 0:64])
                    qrow = sp.tile([1, 64], bf16, tag="qrow")
                    nc.vector.tensor_copy(qrow[:], pq[:])
                    pqr = pb_t([128, 64])
                    nc.tensor.matmul(pqr[:], ones_bf[:], qrow[:],
                                     start=True, stop=True)
                    qrep = sp.tile([128, 64], bf16, tag="qrep")
                    nc.vector.tensor_copy(qrep[:], pqr[:])

                    qb = qrep[:].unsqueeze(1).broadcast_to([128, NT, 64])
                    nc.vector.tensor_mul(eprod[:], onm_bf[:], qb)
                    nc.vector.tensor_add(esc[:, :, 0:32], eprod[:, :, 0:32],
                                         eprod[:, :, 32:64])
                    nc.vector.tensor_add(eprod[:, :, 0:16], esc[:, :, 0:16],
                                         esc[:, :, 16:32])
                    nc.vector.tensor_add(esc[:, :, 0:8], eprod[:, :, 0:8],
                                         eprod[:, :, 8:16])
                    nc.vector.tensor_add(eprod[:, :, 0:4], esc[:, :, 0:4],
                                         esc[:, :, 4:8])
                    nc.vector.tensor_add(esc[:, :, 0:2], eprod[:, :, 0:2],
                                         eprod[:, :, 2:4])
                    e_f = sp.tile([128, NT], f32, tag="e_f")
                    nc.vector.tensor_add(e_f[:], esc[:, :, 0:1].squeeze(2),
                                         esc[:, :, 1:2].squeeze(2))

                    mx = sp.tile([128, 1], f32, tag="mx")
                    nc.vector.tensor_reduce(mx[:], e_f[:], AX.X, ALU.max)
                    mxr = sp.tile([128, 1], f32, tag="mxr")
                    nc.gpsimd.partition_all_reduce(mxr[:], mx[:], 128,
                                                   bass_isa.ReduceOp.max)
                    nmx = sp.tile([128, 1], f32, tag="nmx")
                    nc.scalar.mul(nmx[:], mxr[:], -1.0)
                    att = sp.tile([128, NT], bf16, tag="att")
                    nc.scalar.activation(att[:], e_f[:], AF.Exp, bias=nmx[:])
                    nc.vector.tensor_mul(att[:], att[:], smask_bf[:])
                    sm = sp.tile([128, 1], f32, tag="sm")
                    nc.vector.tensor_reduce(sm[:], att[:], AX.X, ALU.add)
                    smr = sp.tile([128, 1], f32, tag="smr")
                    nc.gpsimd.partition_all_reduce(smr[:], sm[:], 128,
                                                   bass_isa.ReduceOp.add)
                    rs = sp.tile([128, 1], f32, tag="rs")
                    nc.vector.reciprocal(rs[:], smr[:])

                    pr = pb_t([64, 1])
                    for t in range(NT):
                        nc.tensor.matmul(pr[:], onm_bf[:, t, :],
                                         att[:, t:t + 1],
                                         start=(t == 0), stop=(t == NT - 1))
                    nc.vector.tensor_copy(q_lo[:], hs[:])
                    nc.vector.tensor_mul(q_hi[:], pr[:], rs[0:64, :])

                # ---------------- head ----------------
                sh = sp.tile([128, 4, 64], f32, tag="sh")
                nc.gpsimd.dma_gather(sh[:], ag_out[:], hidx_sb[:],
                                     num_idxs=512, num_idxs_reg=512,
                                     elem_size=64, single_packet=False)
                sh_bf = sp.tile([128, 4, 64], bf16, tag="sh_bf")
                nc.vector.tensor_copy(sh_bf[:], sh[:])
                shr = sh_bf[:].rearrange("p (a b) d -> p b a d", b=2)
                shc0 = shr[:, 0:1, :, :].squeeze(1)   # gathered cols 0,2
                shc1 = shr[:, 1:2, :, :].squeeze(1)   # gathered cols 1,3

                psel_bf = stage_cast2(psel, [64, 32], bf16, "psel_bf")
                ql_bf = sp.tile([64, 1], bf16, tag="ql_bf")
                nc.vector.tensor_copy(ql_bf[:], q_lo[:])
                qh_bf = sp.tile([64, 1], bf16, tag="qh_bf")
                nc.vector.tensor_copy(qh_bf[:], q_hi[:])
                # p16 = this core's 16 pool entries
                pp16 = pa_t([16, 1])
                nc.tensor.matmul(pp16[:], psel_bf[:, 0:16], ql_bf[:],
                                 start=True, stop=False)
                nc.tensor.matmul(pp16[:], psel_bf[:, 16:32], qh_bf[:],
                                 start=False, stop=True)
                p16 = sp.tile([16, 1], bf16, tag="p16")
                nc.scalar.copy(p16[:], pp16[:])
                ppr = pa_t([1, 16], bf16)
                nc.tensor.transpose(ppr[:], p16[:], idn_bf[0:16, 0:16])
                p16r = sp.tile([1, 16], bf16, tag="p16r")
                nc.vector.tensor_copy(p16r[:], ppr[:])
                # zrep[k, tau] = pool[16c + tau//8]
                pzr = pb_t([128, 128])
                nc.tensor.matmul(
                    pzr[:], ones_bf[:],
                    p16r[:].unsqueeze(2).broadcast_to([1, 16, 8]),
                    start=True, stop=True)
                zrep = sp.tile([128, 128], bf16, tag="zrep")
                nc.vector.tensor_copy(zrep[:], pzr[:])

                gates = sp.tile([128, 12, 128], f32, tag="gates")
                for mc in [0, 1, 2, 3, 4, 5, 9, 10, 11, 6, 7, 8]:
                    ph = pc_t([128, 128])
                    nc.tensor.matmul(
                        ph[:], wmiT_bf[:, mc * 128:(mc + 1) * 128],
                        shc0, start=True, stop=False)
                    nc.tensor.matmul(
                        ph[:], wmiT_bf[:, 1536 + mc * 128:1536 + (mc + 1) * 128],
                        shc1, start=False, stop=False)
                    nc.tensor.matmul(
                        ph[:], wmiT_bf[:, 3072 + mc * 128:3072 + (mc + 1) * 128],
                        zrep[:], start=False, stop=True)
                    fn = AF.Tanh if mc in (6, 7, 8) else AF.Sigmoid
                    nc.scalar.activation(gates[:, mc, :], ph[:], fn,
                                         bias=mbias_sb[:, mc:mc + 1])
                cm = sp.tile([128, 3, 128], f32, tag="cm")
                nc.vector.tensor_mul(cm[:], gates[:, 0:3, :], gates[:, 6:9, :])
                tcm = sp.tile([128, 3, 128], f32, tag="tcm")
                nc.scalar.activation(tcm[:], cm[:], AF.Tanh)
                hm_bf = sp.tile([128, 3, 128], bf16, tag="hm_bf")
                nc.vector.tensor_mul(hm_bf[:], gates[:, 9:12, :], tcm[:])

                py1 = pc_t([128, 128])
                for kc in range(3):
                    nc.tensor.matmul(py1[:], w1T_bf[:, kc * 128:(kc + 1) * 128],
                                     hm_bf[:, kc, :],
                                     start=(kc == 0), stop=(kc == 2))
                y1_bf = sp.tile([128, 128], bf16, tag="y1_bf")
                nc.scalar.activation(y1_bf[:], py1[:], AF.Relu, bias=b1_sb[:])
                py2 = pc_t([6, 128])
                nc.tensor.matmul(py2[:], w2T_bf[:], y1_bf[:],
                                 start=True, stop=True)
                y_t = sp.tile([6, 128], f32, tag="y_t")
                nc.scalar.activation(y_t[:], py2[:], AF.Identity, bias=b2_sb[:])
                pyt = pc_t([128, 6])
                nc.tensor.transpose(pyt[:], y_t[:], idn_sb[0:6, 0:6])
                y_sb = sp.tile([128, 6], f32, tag="y_sb")
                nc.scalar.copy(y_sb[:], pyt[:])
                nc.sync.dma_start(out_p[:], y_sb[:])

    nc.compile()
    return nc


# --------------------------------------------------------------------------
# host wrapper
# --------------------------------------------------------------------------

def _prepare_inputs(inputs, P):
    d = DIM
    B, nloc, npad = P["B"], P["nloc"], P["npad"]
    E = P["E_shard"]
    NT = npad // 128
    f32 = np.float32

    ea = np.asarray(inputs["edge_attr"], f32)
    We1 = np.asarray(inputs["We1"], f32)
    We2 = np.asarray(inputs["We2"], f32)
    x = np.asarray(inputs["x"], f32)

    jj = np.arange(d * d)
    perm = (jj % d) * d + (jj // d)
    We2p = We2[perm]                          # row j=(f,d)

    node_slot = P["node_slot"]
    NLT = nloc // 128
    # ag row of a padded-global slot: p-major within each core shard
    sl_all = np.arange(npad)
    core_of = sl_all // nloc
    loc_of = sl_all % nloc
    agrow_all = core_of * nloc + (loc_of % 128) * NLT + loc_of // 128
    node_agrow = agrow_all[node_slot]          # orig node -> ag row

    maskrow = np.zeros(npad, f32)
    maskrow[node_agrow] = 1.0
    # onm_bf[p, c*NLT+t] = ag_out[c*nloc + p*NLT + t]
    smask = np.zeros((128, NT), f32)
    cg = (np.arange(NT) // NLT)[None, :]
    tg = (np.arange(NT) % NLT)[None, :]
    pg = np.arange(128)[:, None]
    smask[:, :] = maskrow[cg * nloc + pg * NLT + tg]

    nr = np.asarray(inputs["nonring"]).reshape(-1)
    hlists = np.stack([node_agrow[nr[b::16]] for b in range(16)])

    gb = (np.asarray(inputs["gru_bih"], f32) + np.asarray(inputs["gru_bhh"], f32))
    s2sb = (np.asarray(inputs["s2s_bi"], f32) + np.asarray(inputs["s2s_bh"], f32))
    mbv = (np.asarray(inputs["mem_bi"], f32) + np.asarray(inputs["mem_bh"], f32))
    wiT = np.asarray(inputs["mem_wi"], f32).T          # [384, 1536]
    w1T_ = np.asarray(inputs["W1"], f32).T             # [384, 128]

    shared = {
        "we1_t": np.ascontiguousarray(We1.T),
        "be1": np.asarray(inputs["be1"], f32).reshape(128, 1),
        "we2p_t": np.ascontiguousarray(We2p.T),
        "be2r": np.asarray(inputs["be2"], f32).reshape(d, d),
        "w0m": np.ascontiguousarray(np.asarray(inputs["W0"], f32).T),
        "b0": np.asarray(inputs["b0"], f32).reshape(64, 1),
        "root": np.asarray(inputs["root"], f32),
        "conv_b": np.asarray(inputs["conv_b"], f32).reshape(64, 1),
        "wihT": np.ascontiguousarray(np.asarray(inputs["gru_wih"], f32).T),
        "whhT": np.ascontiguousarray(np.asarray(inputs["gru_whh"], f32).T),
        "grb": gb[0:64].reshape(64, 1),
        "gzb": gb[64:128].reshape(64, 1),
        "bihn": np.asarray(inputs["gru_bih"], f32)[128:192].reshape(64, 1),
        "bhhn": np.asarray(inputs["gru_bhh"], f32)[128:192].reshape(64, 1),
        "s2s_wiT": np.ascontiguousarray(np.concatenate(
            [np.asarray(inputs["s2s_wi"], f32).T[0:64],
             np.asarray(inputs["s2s_wi"], f32).T[64:128]], axis=1)),
        "s2s_whT": np.ascontiguousarray(np.asarray(inputs["s2s_wh"], f32).T),
        "s2s_b": np.ascontiguousarray(s2sb.reshape(4, 64).T),
        "smask": smask,
        "wmiT": np.ascontiguousarray(
            np.concatenate([wiT[0:128], wiT[128:256], wiT[256:384]], axis=1)),
        "mbias": np.ascontiguousarray(mbv.reshape(12, 128).T),
        "w1T": np.ascontiguousarray(
            np.concatenate([w1T_[0:128], w1T_[128:256], w1T_[256:384]], axis=1)),
        "b1": np.asarray(inputs["b1"], f32).reshape(128, 1),
        "w2T": np.ascontiguousarray(np.asarray(inputs["W2"], f32).T),
        "b2": np.asarray(inputs["b2"], f32).reshape(6, 1),
        "idn": np.eye(128, dtype=f32),
    }

    in_maps = []
    for c in range(N_CORES):
        eid = P["edge_id"][c]
        ea_c = np.zeros((E, 7), f32)
        valid = eid >= 0
        ea_c[valid] = ea[eid[valid]]

        x_c = np.zeros((nloc, 3), f32)
        loc = node_slot - c * nloc
        own = (loc >= 0) & (loc < nloc)
        x_c[loc[own]] = x[own]

        ps_c = np.zeros((64, 32), f32)
        for j in range(16):
            k = 16 * c + j
            if k < 64:
                ps_c[k, j] = 1.0
            else:
                ps_c[k - 64, 16 + j] = 1.0

        m = dict(shared)
        m["psel"] = ps_c
        m["ea_t"] = np.ascontiguousarray(ea_c.T)
        m["x_t"] = np.ascontiguousarray(x_c.T)
        m["sel"] = np.ascontiguousarray(P["sel_w"][c].reshape(128, B * VBLK))
        m["gidx"] = _wrap16(agrow_all[P["gather_idx"][c]])
        m["hidx"] = _wrap16(hlists[2 * c:2 * c + 2].reshape(-1))
        in_maps.append(m)
    return in_maps


_CACHE = {}


def _get_built(B, nloc, npad):
    key = (B, nloc, npad)
    if key not in _CACHE:
        _CACHE[key] = _build(B, nloc, npad)
    return _CACHE[key]


def kernel(**inputs) -> np.ndarray:
    from concourse.bass_utils import run_bass_kernel_spmd

    edge_index = np.asarray(inputs["edge_index"])
    n_nodes = np.asarray(inputs["x"]).shape[0]
    P = _host_prep(edge_index, n_nodes)
    in_maps = _prepare_inputs(inputs, P)
    nc = _get_built(P["B"], P["nloc"], P["npad"])
    res = run_bass_kernel_spmd(nc, in_maps, core_ids=list(range(N_CORES)))
    t = np.asarray(inputs["nonring"]).shape[0]
    y = np.zeros((1, t, 6), np.float32)
    for c in range(N_CORES):
        y[0, c * 128:(c + 1) * 128, :] = np.asarray(res.results[c]["out"])
    return y


# revision 18
# speedup vs baseline: 3.9117x; 3.9117x over previous
"""Trainium2 Bass kernel for nn_ActorTorsionNet (GNN message passing).

Sharding: dst-sorted edges packed into 128-edge/48-node-slot blocks; nodes in
8 contiguous shards balanced by edge count. Per-edge [64,64] weights (We)
materialized once to HBM in bf16 with (f,d) column order, streamed back each
of the 6 GRU steps; DVE multiply + add-tree reduce produce messages, and the
scatter-mean is TensorE matmuls against host-built inv_deg-weighted selection
matrices (root/bias terms accumulated in the same PSUM tile). Node features
are d-major [64, nloc]; an AllGather replicates them each step for the edge
gather. Set2Set pooling is replicated; the LSTM head is sharded by torsion.
All per-core differences flow through input tensors (single SPMD graph).
"""
import numpy as np

DIM = 64
N_CORES = 8
EBLK = 128      # edges per block
VBLK = 56       # node slots per block
NCHUNK = 512    # free-dim chunk for node-wise matmuls
ITERS = 6       # GRU iterations (debug knob)


# --------------------------------------------------------------------------
# host-side index prep
# --------------------------------------------------------------------------

def _host_prep(edge_index, n_nodes):
    src = np.asarray(edge_index[0]).astype(np.int64)
    dst = np.asarray(edge_index[1]).astype(np.int64)
    E = src.shape[0]

    deg = np.bincount(dst, minlength=n_nodes)
    inv_deg = (1.0 / np.maximum(deg, 1.0)).astype(np.float32)

    order = np.argsort(dst, kind="stable")
    src_s = src[order]

    ptr = np.zeros(n_nodes + 1, np.int64)
    np.cumsum(deg, out=ptr[1:])

    bounds = [0]
    for c in range(1, N_CORES):
        target = E * c // N_CORES
        bounds.append(int(np.searchsorted(ptr, target, side="left")))
    bounds.append(n_nodes)

    cores = []
    for c in range(N_CORES):
        segs = [(int(v), int(ptr[v]), int(ptr[v + 1]))
                for v in range(bounds[c], bounds[c + 1])]
        segs.sort(key=lambda s: -(s[2] - s[1]))
        blocks = []
        for v, a, b in segs:
            ne = b - a
            for blk in blocks:
                if blk["ne"] + ne <= EBLK and len(blk["segs"]) < VBLK:
                    blk["segs"].append((v, a, b))
                    blk["ne"] += ne
                    break
            else:
                blocks.append({"segs": [(v, a, b)], "ne": ne})
        cores.append(blocks)

    B = max(len(bl) for bl in cores)
    nloc = ((B * VBLK + NCHUNK - 1) // NCHUNK) * NCHUNK
    npad = N_CORES * nloc
    E_shard = B * EBLK

    node_slot = np.full(n_nodes, -1, np.int64)
    edge_id = np.full((N_CORES, E_shard), -1, np.int64)
    gather_src = np.zeros((N_CORES, E_shard), np.int64)
    sel_w = np.zeros((N_CORES, EBLK, B, VBLK), np.float32)
    for c in range(N_CORES):
        for b, blk in enumerate(cores[c]):
            ei = 0
            for nv, (v, a, bb) in enumerate(blk["segs"]):
                node_slot[v] = c * nloc + b * VBLK + nv
                w = inv_deg[v]
                for k in range(a, bb):
                    j = b * EBLK + ei
                    edge_id[c, j] = order[k]
                    gather_src[c, j] = src_s[k]
                    sel_w[c, ei, b, nv] = w
                    ei += 1
    assert (node_slot >= 0).all()
    gather_idx = node_slot[gather_src]

    return dict(B=B, nloc=nloc, npad=npad, E_shard=E_shard, bounds=bounds,
                node_slot=node_slot, edge_id=edge_id, gather_idx=gather_idx,
                sel_w=sel_w)


def _wrap16(idx):
    """dma_gather index layout: value i at [i % 16, i // 16]."""
    idx = np.asarray(idx, np.int64)
    n = idx.shape[0]
    assert n % 16 == 0
    out = np.zeros((16, n // 16), np.int16)
    out[np.arange(n) % 16, np.arange(n) // 16] = idx.astype(np.int16)
    return out


# --------------------------------------------------------------------------
# device kernel builder
# --------------------------------------------------------------------------

def _build(B, nloc, npad):
    import concourse.bacc as bacc
    import concourse.mybir as mybir
    from concourse import bass_isa, tile

    f32 = mybir.dt.float32
    bf16 = mybir.dt.bfloat16
    i16 = mybir.dt.int16
    AF = mybir.ActivationFunctionType
    ALU = mybir.AluOpType
    AX = mybir.AxisListType
    E = B * EBLK
    NLT = nloc // 128
    NT = npad // 128
    NCH = nloc // NCHUNK
    RG = [list(range(N_CORES))]

    nc = bacc.Bacc(debug=False)

    def par(name, shape, dtype=f32):
        return nc.declare_dram_parameter(name, list(shape), dtype, isOutput=False)

    ea_t = par("ea_t", [7, E])
    we1_t = par("we1_t", [7, 128])
    be1 = par("be1", [128, 1])
    we2p_t = par("we2p_t", [128, 4096])
    be2r = par("be2r", [64, 64])
    x_t = par("x_t", [3, nloc])
    w0m = par("w0m", [3, 64])
    b0 = par("b0", [64, 1])
    root_p = par("root", [64, 64])
    conv_b = par("conv_b", [64, 1])
    wihT = par("wihT", [64, 192])
    whhT = par("whhT", [64, 192])
    grb = par("grb", [64, 1])
    gzb = par("gzb", [64, 1])
    bihn = par("bihn", [64, 1])
    bhhn = par("bhhn", [64, 1])
    sel = par("sel", [128, B * VBLK])
    gidx = par("gidx", [16, E // 16], i16)
    hidx = par("hidx", [16, 32], i16)
    s2s_wiT = par("s2s_wiT", [64, 512])
    s2s_whT = par("s2s_whT", [64, 256])
    s2s_b = par("s2s_b", [64, 4])
    smask = par("smask", [128, NT])
    wmiT = par("wmiT", [128, 3 * 1536])
    mbias = par("mbias", [128, 12])
    w1T = par("w1T", [128, 3 * 128])
    b1 = par("b1", [128, 1])
    w2T = par("w2T", [128, 6])
    b2 = par("b2", [6, 1])
    idn = par("idn", [128, 128])
    psel = par("psel", [64, 32])
    out_p = nc.declare_dram_parameter("out", [128, 6], f32, isOutput=True)

    with tile.TileContext(nc) as tc:
        with (
            tc.tile_pool(name="cst", bufs=1) as cst,
            tc.tile_pool(name="st", bufs=1) as st,
            tc.tile_pool(name="psA", bufs=2, space="PSUM") as psA,
            tc.tile_pool(name="psB", bufs=2, space="PSUM") as psB,
            tc.tile_pool(name="psC", bufs=4, space="PSUM") as psC,
            tc.tile_pool(name="dram", bufs=1, space="DRAM") as dram,
        ):
            def pa_t(shape, dtype=f32):
                return psA.tile(list(shape), dtype, tag="psa", name="psa")

            def pb_t(shape, dtype=f32):
                return psB.tile(list(shape), dtype, tag="psb", name="psb")

            def pc_t(shape, dtype=f32):
                return psC.tile(list(shape), dtype, tag="psc", name="psc")

            # ----- persistent constants (small, bf16 where possible) -----
            def loadc(pool, p, shape, dtype=f32, tag=None):
                t = pool.tile(list(shape), dtype, tag=tag or ("ld_" + p.name), name="ld_" + p.name)
                nc.sync.dma_start(t[:], p[:])
                return t

            be1_sb = loadc(cst, be1, [128, 1])
            b0_sb = loadc(cst, b0, [64, 1])
            conv_sb = loadc(cst, conv_b, [64, 1])
            grb_sb = loadc(cst, grb, [64, 1])
            gzb_sb = loadc(cst, gzb, [64, 1])
            bihn_sb = loadc(cst, bihn, [64, 1])
            bhhn_sb = loadc(cst, bhhn, [64, 1])
            gidx_sb = cst.tile([128, E // 16], i16, tag="gidx_sb")
            nc.vector.memset(gidx_sb[:], 0)
            nc.sync.dma_start(gidx_sb[0:16, :], gidx[:])
            idn_sb = loadc(cst, idn, [128, 128])
            idn_bf = cst.tile([128, 128], bf16, tag="idn_bf")
            nc.vector.tensor_copy(idn_bf[:], idn_sb[:])
            ones_bf = cst.tile([1, 128], bf16, tag="ones_bf")
            nc.vector.memset(ones_bf[:], 1.0)
            ones_f = cst.tile([1, 128], f32, tag="ones_f")
            nc.vector.memset(ones_f[:], 1.0)

            sel_bf = cst.tile([128, B * VBLK], bf16, tag="sel_bf")
            root_bf = cst.tile([64, 64], bf16, tag="root_bf")
            be2r_bf = cst.tile([64, 64], bf16, tag="be2r_bf")
            wihT_bf = cst.tile([64, 192], bf16, tag="wihT_bf")
            whhT_bf = cst.tile([64, 192], bf16, tag="whhT_bf")

            # ----- persistent state -----
            h_t = st.tile([64, nloc], f32, tag="h_t")
            nc.vector.memset(h_t[:], 0.0)
            out_bf = st.tile([64, nloc], bf16, tag="out_bf")
            m_bf = st.tile([64, nloc], bf16, tag="m_bf")
            nc.vector.memset(m_bf[:], 0.0)
            s_gat = st.tile([128, B, 64], f32, tag="s_gat")
            s_bf = st.tile([128, B, 64], bf16, tag="s_bf")

            # ----- internal DRAM -----
            we_dram = dram.tile([E, 4096], bf16)
            ag_ins = []
            ag_outs = []
            for i in range(ITERS + 1):
                agi = dram.tile([nloc, 64], f32, name=f"ag_in{i}", tag=f"agi{i}")
                ago = dram.tile([npad, 64], f32, addr_space="Shared",
                                name=f"ag_out{i}", tag=f"ago{i}")
                ag_ins.append(agi)
                ag_outs.append(ago)

            # =============== phase 0: staging + edge MLP -> We ===============
            with tc.tile_pool(name="ph0", bufs=1) as ph0:
                def stage_cast(p, shape, dst):
                    t = ph0.tile(list(shape), f32, tag="stage", name="stage")
                    nc.sync.dma_start(t[:], p[:])
                    nc.vector.tensor_copy(dst[:], t[:])

                stage_cast(sel, [128, B * VBLK], sel_bf)
                stage_cast(root_p, [64, 64], root_bf)
                stage_cast(be2r, [64, 64], be2r_bf)
                stage_cast(wihT, [64, 192], wihT_bf)
                stage_cast(whhT, [64, 192], whhT_bf)

                ea_sb = loadc(ph0, ea_t, [7, E])
                we1_sb = loadc(ph0, we1_t, [7, 128])
                x_sb = loadc(ph0, x_t, [3, nloc])
                w0m_sb = loadc(ph0, w0m, [3, 64])
                we2p_sb = loadc(ph0, we2p_t, [128, 4096])
                we2p_bf = ph0.tile([128, 4096], bf16, tag="we2p_bf")
                nc.vector.tensor_copy(we2p_bf[:], we2p_sb[:])

                h_edge = ph0.tile([128, E], bf16, tag="h_edge")
                for ch in range(E // NCHUNK):
                    ps = pc_t([128, NCHUNK])
                    nc.tensor.matmul(ps[:], we1_sb[:],
                                     ea_sb[:, ch * NCHUNK:(ch + 1) * NCHUNK],
                                     start=True, stop=True)
                    nc.scalar.activation(h_edge[:, ch * NCHUNK:(ch + 1) * NCHUNK],
                                         ps[:], AF.Relu, bias=be1_sb[:])

                with tc.tile_pool(name="wes", bufs=2) as wes:
                    for b in range(B):
                        we_sb = wes.tile([128, 4096], bf16, tag="wesb")
                        for j in range(8):
                            ps = pc_t([128, 512])
                            nc.tensor.matmul(
                                ps[:], h_edge[:, b * 128:(b + 1) * 128],
                                we2p_bf[:, j * 512:(j + 1) * 512],
                                start=True, stop=True)
                            if j % 2 == 0:
                                nc.vector.tensor_copy(
                                    we_sb[:, j * 512:(j + 1) * 512], ps[:])
                            else:
                                nc.scalar.copy(
                                    we_sb[:, j * 512:(j + 1) * 512], ps[:])
                        nc.sync.dma_start(we_dram[b * 128:(b + 1) * 128, :],
                                          we_sb[:])

                # lin0 (inside ph0 scope: uses x_sb)
                for ch in range(NCH):
                    sl = slice(ch * NCHUNK, (ch + 1) * NCHUNK)
                    ps = pc_t([64, NCHUNK])
                    nc.tensor.matmul(ps[:], w0m_sb[:], x_sb[:, sl],
                                     start=True, stop=True)
                    nc.scalar.activation(h_t[:, sl], ps[:], AF.Relu, bias=b0_sb[:])

            # =============== 6 GRU iterations ===============
            with (
                tc.tile_pool(name="wk", bufs=3) as wk,
                tc.tile_pool(name="tr", bufs=2) as tr,
            ):
                def emit_allgather(ag_in, ag_out):
                    onm = wk.tile([128, NLT * 64], f32, tag="onm")
                    for t0 in range(0, NLT, 8):
                        grp = min(8, NLT - t0)
                        ps = pc_t([128, 512])
                        for k in range(grp):
                            t = t0 + k
                            nc.tensor.transpose(ps[:, k * 64:(k + 1) * 64],
                                                h_t[:, t * 128:(t + 1) * 128],
                                                idn_sb[0:64, 0:64])
                        nc.scalar.copy(onm[:, t0 * 64:(t0 + grp) * 64],
                                       ps[:, 0:grp * 64])
                    nc.sync.dma_start(
                        ag_in[:].rearrange("(p t) d -> p t d", p=128),
                        onm[:].rearrange("p (t d) -> p t d", d=64))
                    nc.gpsimd.collective_compute(
                        "AllGather", ALU.bypass, replica_groups=RG,
                        ins=[ag_in[:].opt()], outs=[ag_out[:].opt()])

                def tree_reduce(eng, tmp, sc1, F, msg_out):
                    """Ping-pong pairwise adds over trailing 64 -> msg_out."""
                    nc_e = eng
                    nc_e.tensor_add(sc1[:, :, 0:32], tmp[:, :, 0:32],
                                    tmp[:, :, 32:64])
                    nc_e.tensor_add(tmp[:, :, 0:16], sc1[:, :, 0:16],
                                    sc1[:, :, 16:32])
                    nc_e.tensor_add(sc1[:, :, 0:8], tmp[:, :, 0:8],
                                    tmp[:, :, 8:16])
                    nc_e.tensor_add(tmp[:, :, 0:4], sc1[:, :, 0:4],
                                    sc1[:, :, 4:8])
                    nc_e.tensor_add(sc1[:, :, 0:2], tmp[:, :, 0:2],
                                    tmp[:, :, 2:4])
                    nc_e.tensor_add(msg_out[:], sc1[:, :, 0:1].squeeze(2),
                                    sc1[:, :, 1:2].squeeze(2))

                for it in range(ITERS):
                    ag_out = ag_outs[it]
                    nc.vector.tensor_copy(out_bf[:], h_t[:])
                    emit_allgather(ag_ins[it], ag_out)
                    nc.gpsimd.dma_gather(s_gat[:], ag_out[:], gidx_sb[:],
                                         num_idxs=E, num_idxs_reg=E,
                                         elem_size=64, single_packet=False)
                    nc.vector.tensor_copy(s_bf[:], s_gat[:])

                    for b in range(B):
                        we_sb = wk.tile([128, 64, 64], bf16, tag="westream")
                        nc.sync.dma_start(
                            we_sb[:].rearrange("p a b -> p (a b)"),
                            we_dram[b * 128:(b + 1) * 128, :])
                        tmp = tr.tile([128, 64, 64], bf16, tag="tmp")
                        sc1 = tr.tile([128, 64, 32], bf16, tag="sc1")
                        sb_b = s_bf[:, b, :].unsqueeze(1).broadcast_to(
                            [128, 64, 64])
                        nc.vector.tensor_mul(tmp[:], we_sb[:], sb_b)
                        msg = tr.tile([128, 64], bf16, tag="msg")
                        tree_reduce(nc.vector, tmp, sc1, 64, msg)

                        vsl = slice(b * VBLK, (b + 1) * VBLK)
                        pa = pa_t([64, VBLK])
                        nc.tensor.matmul(pa[:], s_bf[:, b, :], sel_bf[:, vsl],
                                         start=True, stop=True)
                        ssum_bf = wk.tile([64, VBLK], bf16, tag="ssum")
                        nc.scalar.copy(ssum_bf[:], pa[:])
                        pb = pb_t([64, VBLK])
                        nc.tensor.matmul(pb[:], be2r_bf[:], ssum_bf[:],
                                         start=True, stop=False)
                        nc.tensor.matmul(pb[:], msg[:], sel_bf[:, vsl],
                                         start=False, stop=False)
                        nc.tensor.matmul(pb[:], root_bf[:], out_bf[:, vsl],
                                         start=False, stop=True)
                        nc.scalar.activation(m_bf[:, vsl], pb[:], AF.Relu,
                                             bias=conv_sb[:])

                    for ch in range(NCH):
                        sl = slice(ch * NCHUNK, (ch + 1) * NCHUNK)
                        pr_ = pc_t([64, NCHUNK])
                        nc.tensor.matmul(pr_[:], wihT_bf[:, 0:64], m_bf[:, sl],
                                         start=True, stop=False)
                        nc.tensor.matmul(pr_[:], whhT_bf[:, 0:64],
                                         out_bf[:, sl], start=False, stop=True)
                        pz_ = pc_t([64, NCHUNK])
                        nc.tensor.matmul(pz_[:], wihT_bf[:, 64:128],
                                         m_bf[:, sl], start=True, stop=False)
                        nc.tensor.matmul(pz_[:], whhT_bf[:, 64:128],
                                         out_bf[:, sl], start=False, stop=True)
                        pxn = pc_t([64, NCHUNK])
                        nc.tensor.matmul(pxn[:], wihT_bf[:, 128:192],
                                         m_bf[:, sl], start=True, stop=True)
                        phn = pc_t([64, NCHUNK])
                        nc.tensor.matmul(phn[:], whhT_bf[:, 128:192],
                                         out_bf[:, sl], start=True, stop=True)
                        r_sb = wk.tile([64, NCHUNK], f32, tag="r_sb")
                        nc.scalar.activation(r_sb[:], pr_[:], AF.Sigmoid,
                                             bias=grb_sb[:])
                        z_sb = wk.tile([64, NCHUNK], f32, tag="z_sb")
                        nc.scalar.activation(z_sb[:], pz_[:], AF.Sigmoid,
                                             bias=gzb_sb[:])
                        ghn = wk.tile([64, NCHUNK], f32, tag="ghn")
                        nc.scalar.activation(ghn[:], phn[:], AF.Identity,
                                             bias=bhhn_sb[:])
                        t1 = wk.tile([64, NCHUNK], f32, tag="t1")
                        nc.vector.tensor_mul(t1[:], r_sb[:], ghn[:])
                        t2 = wk.tile([64, NCHUNK], f32, tag="t2")
                        nc.vector.tensor_add(t2[:], t1[:], pxn[:])
                        nn = wk.tile([64, NCHUNK], f32, tag="nn")
                        nc.scalar.activation(nn[:], t2[:], AF.Tanh,
                                             bias=bihn_sb[:])
                        t3 = wk.tile([64, NCHUNK], f32, tag="t3")
                        nc.vector.tensor_sub(t3[:], h_t[:, sl], nn[:])
                        t4 = wk.tile([64, NCHUNK], f32, tag="t4")
                        nc.vector.tensor_mul(t4[:], z_sb[:], t3[:])
                        nc.vector.tensor_add(h_t[:, sl], nn[:], t4[:])

                emit_allgather(ag_ins[ITERS], ag_outs[ITERS])

            # =============== Set2Set + head ===============
            with tc.tile_pool(name="s2s", bufs=1) as sp:
                def stage_cast2(p, shape, dtype, tag):
                    t = sp.tile(list(shape), f32, tag="stage2", name="stage2")
                    nc.sync.dma_start(t[:], p[:])
                    o = sp.tile(list(shape), dtype, tag=tag, name=tag)
                    nc.vector.tensor_copy(o[:], t[:])
                    return o

                s2s_wiT_bf = stage_cast2(s2s_wiT, [64, 512], bf16, "wi_bf")
                s2s_whT_bf = stage_cast2(s2s_whT, [64, 256], bf16, "wh_bf")
                smask_bf = stage_cast2(smask, [128, NT], bf16, "smask_bf")
                wmiT_bf = stage_cast2(wmiT, [128, 3 * 1536], bf16, "wmiT_bf")
                w1T_bf = stage_cast2(w1T, [128, 3 * 128], bf16, "w1T_bf")
                w2T_bf = stage_cast2(w2T, [128, 6], bf16, "w2T_bf")
                s2s_b_sb = loadc(sp, s2s_b, [64, 4])
                mbias_sb = loadc(sp, mbias, [128, 12])
                b1_sb = loadc(sp, b1, [128, 1])
                b2_sb = loadc(sp, b2, [6, 1])
                hidx_sb = sp.tile([128, 32], i16, tag="hidx_sb")
                nc.vector.memset(hidx_sb[:], 0)
                nc.sync.dma_start(hidx_sb[0:16, :], hidx[:])

                ag_out = ag_outs[ITERS]
                onm_bf = sp.tile([128, NT, 64], bf16, tag="onm_bf")
                nc.gpsimd.dma_start(
                    onm_bf[:].rearrange("p (c t) d -> p c t d", c=N_CORES),
                    ag_out[:].rearrange("(c p t) d -> p c t d",
                                        c=N_CORES, p=128))
                mb = smask_bf[:].unsqueeze(2).broadcast_to([128, NT, 64])
                nc.vector.tensor_mul(onm_bf[:], onm_bf[:], mb)

                q_lo = sp.tile([64, 1], f32, tag="q_lo")
                nc.vector.memset(q_lo[:], 0.0)
                q_hi = sp.tile([64, 1], f32, tag="q_hi")
                nc.vector.memset(q_hi[:], 0.0)
                hs = sp.tile([64, 1], f32, tag="hs")
                nc.vector.memset(hs[:], 0.0)
                cs = sp.tile([64, 1], f32, tag="cs")
                nc.vector.memset(cs[:], 0.0)

                eprod = sp.tile([128, NT, 64], bf16, tag="eprod")
                esc = sp.tile([128, NT, 32], bf16, tag="esc")

                for step in range(6):
                    ql_bf = sp.tile([64, 1], bf16, tag="ql_bf")
                    nc.vector.tensor_copy(ql_bf[:], q_lo[:])
                    qh_bf = sp.tile([64, 1], bf16, tag="qh_bf")
                    nc.vector.tensor_copy(qh_bf[:], q_hi[:])
                    hs_bf = sp.tile([64, 1], bf16, tag="hs_bf")
                    nc.vector.tensor_copy(hs_bf[:], hs[:])
                    gt = []
                    for g, fn in enumerate([AF.Sigmoid, AF.Sigmoid,
                                            AF.Tanh, AF.Sigmoid]):
                        pg = pa_t([64, 1])
                        gsl = slice(g * 64, (g + 1) * 64)
                        nc.tensor.matmul(pg[:], s2s_wiT_bf[:, gsl], ql_bf[:],
                                         start=True, stop=False)
                        nc.tensor.matmul(pg[:],
                                         s2s_wiT_bf[:, 256 + g * 64:
                                                    256 + (g + 1) * 64],
                                         qh_bf[:], start=False, stop=False)
                        nc.tensor.matmul(pg[:], s2s_whT_bf[:, gsl], hs_bf[:],
                                         start=False, stop=True)
                        gv = sp.tile([64, 1], f32, tag=f"gate{g}", name=f"gate{g}")
                        nc.scalar.activation(gv[:], pg[:], fn,
                                             bias=s2s_b_sb[:, g:g + 1])
                        gt.append(gv)
                    t5 = sp.tile([64, 1], f32, tag="t5")
                    nc.vector.tensor_mul(t5[:], gt[1][:], cs[:])
                    t6 = sp.tile([64, 1], f32, tag="t6")
                    nc.vector.tensor_mul(t6[:], gt[0][:], gt[2][:])
                    nc.vector.tensor_add(cs[:], t5[:], t6[:])
                    tch = sp.tile([64, 1], f32, tag="tch")
                    nc.scalar.activation(tch[:], cs[:], AF.Tanh)
                    nc.vector.tensor_mul(hs[:], gt[3][:], tch[:])

                    hsb2 = sp.tile([64, 1], bf16, tag="hsb2")
                    nc.vector.tensor_copy(hsb2[:], hs[:])
                    pq = pa_t([1, 64], bf16)
                    nc.tensor.transpose(pq[:], hsb2[:], idn_bf[0:64, 0:64])
                    qrow = sp.tile([1, 64], bf16, tag="qrow")
                    nc.vector.tensor_copy(qrow[:], pq[:])
                    pqr = pb_t([128, 64])
                    nc.tensor.matmul(pqr[:], ones_bf[:], qrow[:],
                                     start=True, stop=True)
                    qrep = sp.tile([128, 64], bf16, tag="qrep")
                    nc.vector.tensor_copy(qrep[:], pqr[:])

                    qb = qrep[:].unsqueeze(1).broadcast_to([128, NT, 64])
                    nc.vector.tensor_mul(eprod[:], onm_bf[:], qb)
                    nc.vector.tensor_add(esc[:, :, 0:32], eprod[:, :, 0:32],
                                         eprod[:, :, 32:64])
                    nc.vector.tensor_add(eprod[:, :, 0:16], esc[:, :, 0:16],
                                         esc[:, :, 16:32])
                    nc.vector.tensor_add(esc[:, :, 0:8], eprod[:, :, 0:8],
                                         eprod[:, :, 8:16])
                    nc.vector.tensor_add(eprod[:, :, 0:4], esc[:, :, 0:4],
                                         esc[:, :, 4:8])
                    nc.vector.tensor_add(esc[:, :, 0:2], eprod[:, :, 0:2],
                                         eprod[:, :, 2:4])
                    e_f = sp.tile([128, NT], f32, tag="e_f")
                    nc.vector.tensor_add(e_f[:], esc[:, :, 0:1].squeeze(2),
                                         esc[:, :, 1:2].squeeze(2))

                    mx = sp.tile([128, 1], f32, tag="mx")
                    nc.vector.tensor_reduce(mx[:], e_f[:], AX.X, ALU.max)
                    mxr = sp.tile([128, 1], f32, tag="mxr")
                    nc.gpsimd.partition_all_reduce(mxr[:], mx[:], 128,
                                                   bass_isa.ReduceOp.max)
                    nmx = sp.tile([128, 1], f32, tag="nmx")
                    nc.scalar.mul(nmx[:], mxr[:], -1.0)
                    att = sp.tile([128, NT], bf16, tag="att")
                    nc.scalar.activation(att[:], e_f[:], AF.Exp, bias=nmx[:])
                    nc.vector.tensor_mul(att[:], att[:], smask_bf[:])
                    sm = sp.tile([128, 1], f32, tag="sm")
                    nc.vector.tensor_reduce(sm[:], att[:], AX.X, ALU.add)
                    smr = sp.tile([128, 1], f32, tag="smr")
                    nc.gpsimd.partition_all_reduce(smr[:], sm[:], 128,
                                                   bass_isa.ReduceOp.add)
                    rs = sp.tile([128, 1], f32, tag="rs")
                    nc.vector.reciprocal(rs[:], smr[:])

                    pr = pb_t([64, 1])
                    for t in range(NT):
                        nc.tensor.matmul(pr[:], onm_bf[:, t, :],
                                         att[:, t:t + 1],
                                         start=(t == 0), stop=(t == NT - 1))
                    nc.vector.tensor_copy(q_lo[:], hs[:])
                    nc.vector.tensor_mul(q_hi[:], pr[:], rs[0:64, :])

                # ---------------- head ----------------
                sh = sp.tile([128, 4, 64], f32, tag="sh")
                nc.gpsimd.dma_gather(sh[:], ag_out[:], hidx_sb[:],
                                     num_idxs=512, num_idxs_reg=512,
                                     elem_size=64, single_packet=False)
                sh_bf = sp.tile([128, 4, 64], bf16, tag="sh_bf")
                nc.vector.tensor_copy(sh_bf[:], sh[:])
                shr = sh_bf[:].rearrange("p (a b) d -> p b a d", b=2)
                shc0 = shr[:, 0:1, :, :].squeeze(1)   # gathered cols 0,2
                shc1 = shr[:, 1:2, :, :].squeeze(1)   # gathered cols 1,3

                psel_bf = stage_cast2(psel, [64, 32], bf16, "psel_bf")
                ql_bf = sp.tile([64, 1], bf16, tag="ql_bf")
                nc.vector.tensor_copy(ql_bf[:], q_lo[:])
                qh_bf = sp.tile([64, 1], bf16, tag="qh_bf")
                nc.vector.tensor_copy(qh_bf[:], q_hi[:])
                # p16 = this core's 16 pool entries
                pp16 = pa_t([16, 1])
                nc.tensor.matmul(pp16[:], psel_bf[:, 0:16], ql_bf[:],
                                 start=True, stop=False)
                nc.tensor.matmul(pp16[:], psel_bf[:, 16:32], qh_bf[:],
                                 start=False, stop=True)
                p16 = sp.tile([16, 1], bf16, tag="p16")
                nc.scalar.copy(p16[:], pp16[:])
                ppr = pa_t([1, 16], bf16)
                nc.tensor.transpose(ppr[:], p16[:], idn_bf[0:16, 0:16])
                p16r = sp.tile([1, 16], bf16, tag="p16r")
                nc.vector.tensor_copy(p16r[:], ppr[:])
                # zrep[k, tau] = pool[16c + tau//8]
                pzr = pb_t([128, 128])
                nc.tensor.matmul(
                    pzr[:], ones_bf[:],
                    p16r[:].unsqueeze(2).broadcast_to([1, 16, 8]),
                    start=True, stop=True)
                zrep = sp.tile([128, 128], bf16, tag="zrep")
                nc.vector.tensor_copy(zrep[:], pzr[:])

                gates = sp.tile([128, 12, 128], f32, tag="gates")
                for mc in [0, 1, 2, 3, 4, 5, 9, 10, 11, 6, 7, 8]:
                    ph = pc_t([128, 128])
                    nc.tensor.matmul(
                        ph[:], wmiT_bf[:, mc * 128:(mc + 1) * 128],
                        shc0, start=True, stop=False)
                    nc.tensor.matmul(
                        ph[:], wmiT_bf[:, 1536 + mc * 128:1536 + (mc + 1) * 128],
                        shc1, start=False, stop=False)
                    nc.tensor.matmul(
                        ph[:], wmiT_bf[:, 3072 + mc * 128:3072 + (mc + 1) * 128],
                        zrep[:], start=False, stop=True)
                    fn = AF.Tanh if mc in (6, 7, 8) else AF.Sigmoid
                    nc.scalar.activation(gates[:, mc, :], ph[:], fn,
                                         bias=mbias_sb[:, mc:mc + 1])
                cm = sp.tile([128, 3, 128], f32, tag="cm")
                nc.vector.tensor_mul(cm[:], gates[:, 0:3, :], gates[:, 6:9, :])
                tcm = sp.tile([128, 3, 128], f32, tag="tcm")
                nc.scalar.activation(tcm[:], cm[:], AF.Tanh)
                hm_bf = sp.tile([128, 3, 128], bf16, tag="hm_bf")
                nc.vector.tensor_mul(hm_bf[:], gates[:, 9:12, :], tcm[:])

                py1 = pc_t([128, 128])
                for kc in range(3):
                    nc.tensor.matmul(py1[:], w1T_bf[:, kc * 128:(kc + 1) * 128],
                                     hm_bf[:, kc, :],
                                     start=(kc == 0), stop=(kc == 2))
                y1_bf = sp.tile([128, 128], bf16, tag="y1_bf")
                nc.scalar.activation(y1_bf[:], py1[:], AF.Relu, bias=b1_sb[:])
                py2 = pc_t([6, 128])
                nc.tensor.matmul(py2[:], w2T_bf[:], y1_bf[:],
                                 start=True, stop=True)
                y_t = sp.tile([6, 128], f32, tag="y_t")
                nc.scalar.activation(y_t[:], py2[:], AF.Identity, bias=b2_sb[:])
                pyt = pc_t([128, 6])
                nc.tensor.transpose(pyt[:], y_t[:], idn_sb[0:6, 0:6])
                y_sb = sp.tile([128, 6], f32, tag="y_sb")
                nc.scalar.copy(y_sb[:], pyt[:])
                nc.sync.dma_start(out_p[:], y_sb[:])

    nc.compile()
    return nc


# --------------------------------------------------------------------------
# host wrapper
# --------------------------------------------------------------------------

def _prepare_inputs(inputs, P):
    d = DIM
    B, nloc, npad = P["B"], P["nloc"], P["npad"]
    E = P["E_shard"]
    NT = npad // 128
    f32 = np.float32

    ea = np.asarray(inputs["edge_attr"], f32)
    We1 = np.asarray(inputs["We1"], f32)
    We2 = np.asarray(inputs["We2"], f32)
    x = np.asarray(inputs["x"], f32)

    jj = np.arange(d * d)
    perm = (jj % d) * d + (jj // d)
    We2p = We2[perm]                          # row j=(f,d)

    node_slot = P["node_slot"]
    NLT = nloc // 128
    # ag row of a padded-global slot: p-major within each core shard
    sl_all = np.arange(npad)
    core_of = sl_all // nloc
    loc_of = sl_all % nloc
    agrow_all = core_of * nloc + (loc_of % 128) * NLT + loc_of // 128
    node_agrow = agrow_all[node_slot]          # orig node -> ag row

    maskrow = np.zeros(npad, f32)
    maskrow[node_agrow] = 1.0
    # onm_bf[p, c*NLT+t] = ag_out[c*nloc + p*NLT + t]
    smask = np.zeros((128, NT), f32)
    cg = (np.arange(NT) // NLT)[None, :]
    tg = (np.arange(NT) % NLT)[None, :]
    pg = np.arange(128)[:, None]
    smask[:, :] = maskrow[cg * nloc + pg * NLT + tg]

    nr = np.asarray(inputs["nonring"]).reshape(-1)
    hlists = np.stack([node_agrow[nr[b::16]] for b in range(16)])

    gb = (np.asarray(inputs["gru_bih"], f32) + np.asarray(inputs["gru_bhh"], f32))
    s2sb = (np.asarray(inputs["s2s_bi"], f32) + np.asarray(inputs["s2s_bh"], f32))
    mbv = (np.asarray(inputs["mem_bi"], f32) + np.asarray(inputs["mem_bh"], f32))
    wiT = np.asarray(inputs["mem_wi"], f32).T          # [384, 1536]
    w1T_ = np.asarray(inputs["W1"], f32).T             # [384, 128]

    shared = {
        "we1_t": np.ascontiguousarray(We1.T),
        "be1": np.asarray(inputs["be1"], f32).reshape(128, 1),
        "we2p_t": np.ascontiguousarray(We2p.T),
        "be2r": np.asarray(inputs["be2"], f32).reshape(d, d),
        "w0m": np.ascontiguousarray(np.asarray(inputs["W0"], f32).T),
        "b0": np.asarray(inputs["b0"], f32).reshape(64, 1),
        "root": np.asarray(inputs["root"], f32),
        "conv_b": np.asarray(inputs["conv_b"], f32).reshape(64, 1),
        "wihT": np.ascontiguousarray(np.asarray(inputs["gru_wih"], f32).T),
        "whhT": np.ascontiguousarray(np.asarray(inputs["gru_whh"], f32).T),
        "grb": gb[0:64].reshape(64, 1),
        "gzb": gb[64:128].reshape(64, 1),
        "bihn": np.asarray(inputs["gru_bih"], f32)[128:192].reshape(64, 1),
        "bhhn": np.asarray(inputs["gru_bhh"], f32)[128:192].reshape(64, 1),
        "s2s_wiT": np.ascontiguousarray(np.concatenate(
            [np.asarray(inputs["s2s_wi"], f32).T[0:64],
             np.asarray(inputs["s2s_wi"], f32).T[64:128]], axis=1)),
        "s2s_whT": np.ascontiguousarray(np.asarray(inputs["s2s_wh"], f32).T),
        "s2s_b": np.ascontiguousarray(s2sb.reshape(4, 64).T),
        "smask": smask,
        "wmiT": np.ascontiguousarray(
            np.concatenate([wiT[0:128], wiT[128:256], wiT[256:384]], axis=1)),
        "mbias": np.ascontiguousarray(mbv.reshape(12, 128).T),
        "w1T": np.ascontiguousarray(
            np.concatenate([w1T_[0:128], w1T_[128:256], w1T_[256:384]], axis=1)),
        "b1": np.asarray(inputs["b1"], f32).reshape(128, 1),
        "w2T": np.ascontiguousarray(np.asarray(inputs["W2"], f32).T),
        "b2": np.asarray(inputs["b2"], f32).reshape(6, 1),
        "idn": np.eye(128, dtype=f32),
    }

    in_maps = []
    for c in range(N_CORES):
        eid = P["edge_id"][c]
        ea_c = np.zeros((E, 7), f32)
        valid = eid >= 0
        ea_c[valid] = ea[eid[valid]]

        x_c = np.zeros((nloc, 3), f32)
        loc = node_slot - c * nloc
        own = (loc >= 0) & (loc < nloc)
        x_c[loc[own]] = x[own]

        ps_c = np.zeros((64, 32), f32)
        for j in range(16):
            k = 16 * c + j
            if k < 64:
                ps_c[k, j] = 1.0
            else:
                ps_c[k - 64, 16 + j] = 1.0

        m = dict(shared)
        m["psel"] = ps_c
        m["ea_t"] = np.ascontiguousarray(ea_c.T)
        m["x_t"] = np.ascontiguousarray(x_c.T)
        m["sel"] = np.ascontiguousarray(P["sel_w"][c].reshape(128, B * VBLK))
        m["gidx"] = _wrap16(agrow_all[P["gather_idx"][c]])
        m["hidx"] = _wrap16(hlists[2 * c:2 * c + 2].reshape(-1))
        in_maps.append(m)
    return in_maps


_CACHE = {}


def _get_built(B, nloc, npad):
    key = (B, nloc, npad)
    if key not in _CACHE:
        _CACHE[key] = _build(B, nloc, npad)
    return _CACHE[key]


def kernel(**inputs) -> np.ndarray:
    from concourse.bass_utils import run_bass_kernel_spmd

    edge_index = np.asarray(inputs["edge_index"])
    n_nodes = np.asarray(inputs["x"]).shape[0]
    P = _host_prep(edge_index, n_nodes)
    in_maps = _prepare_inputs(inputs, P)
    nc = _get_built(P["B"], P["nloc"], P["npad"])
    res = run_bass_kernel_spmd(nc, in_maps, core_ids=list(range(N_CORES)))
    t = np.asarray(inputs["nonring"]).shape[0]
    y = np.zeros((1, t, 6), np.float32)
    for c in range(N_CORES):
        y[0, c * 128:(c + 1) * 128, :] = np.asarray(res.results[c]["out"])
    return y
